# revision 24
# baseline (speedup 1.0000x reference)
"""LSTM (CustomRNNLayer) Trainium2 Bass kernel.

Strategy:
  - Data-parallel over batch: 64 sequences -> 8 cores x B=8.
  - Host pre-transposes x into time-major, feature-on-partition layout
    (xT[k][p, s*8+b]) so the device never transposes anything.
  - Phase 1 (parallel): X-projection GEMM for all timesteps,
    Xp = x @ Wx.T + bias, stored gate-major ([128 gate-partitions,
    t*128 + gc*8 + b] layout) in scratch DRAM.
  - Phase 2 (serial recurrence): per step t, 64 weight-stationary
    matmuls (lhsT = WhT chunk [128,128] bf16 -> FWL, rhs = hT [128,8])
    accumulate G.T in one PSUM tile [128,128]; DVE adds Xp slice; ACT
    sigmoid/tanh (single table set); DVE updates c,h; h written to the
    output window (fp32) and converted to bf16 for the next matmul.
  - Output written as [128, S*32] (cols t*32 + hc*8 + b), reassembled
    on host. h_last = outputs[:, -1].
"""

from contextlib import ExitStack

import ml_dtypes
import numpy as np

import concourse.bass as bass
import concourse.mybir as mybir
import concourse.tile as tile
from concourse import bacc
from concourse.bass_utils import run_bass_kernel_spmd
from concourse.masks import make_identity

NCORES = 8
B = 8  # batch per core
D = 512  # input size
H = 512  # hidden size
G4 = 4 * H  # 2048 gate rows (f, i, o, c order)
KC = D // 128  # 4 contraction chunks
GC = G4 // 128  # 16 gate chunks
WIN = 64  # timesteps per window

F32 = mybir.dt.float32
BF16 = mybir.dt.bfloat16
AF = mybir.ActivationFunctionType

import os as _os

_DBG_SKIP_EW = bool(_os.environ.get("LSTM_DBG_SKIP_EW"))
_DBG_SKIP_MM = bool(_os.environ.get("LSTM_DBG_SKIP_MM"))
_N_STREAMS = int(_os.environ.get("LSTM_STREAMS", "1"))
_STREAM_SPLIT = (
    [(0, B)] if _N_STREAMS == 1 else [(0, B // 2), (B // 2, B // 2)]
)

_cache: dict = {}
LAST_RESULTS = None  # test harness reads exec_time_ns from here
LAST_EXEC_WALL = None  # wall seconds of the SPMD run (test harness)


def _build(S: int) -> bass.Bass:
    NW = S // WIN
    assert S % WIN == 0

    nc = bacc.Bacc("TRN2", debug=False, enable_asserts=False, num_devices=NCORES)
    xT = nc.dram_tensor("xT", [KC, 128, S * B], BF16, kind="ExternalInput").ap()
    wxT = nc.dram_tensor("wxT", [KC, 128, G4], BF16, kind="ExternalInput").ap()
    whT = nc.dram_tensor("whT", [KC, 128, G4], BF16, kind="ExternalInput").ap()
    bias = nc.dram_tensor("bias", [128, GC], F32, kind="ExternalInput").ap()
    out = nc.dram_tensor("out", [128, S * 32], F32, kind="ExternalOutput").ap()

    with tile.TileContext(nc) as tc, ExitStack() as ctx:
        singles = ctx.enter_context(tc.tile_pool(name="singles", bufs=1))
        dram_pool = ctx.enter_context(tc.tile_pool(name="dram", bufs=1, space="DRAM"))

        # scratch DRAM for the x-projection (gate-major step-block layout)
        xp_dram = dram_pool.tile([128, S * 128], BF16)

        # resident weights
        wx_sb = singles.tile([128, KC, G4], BF16)
        wh_sb = singles.tile([128, KC, G4], BF16)
        bias_sb = singles.tile([128, GC], F32)
        for k in range(KC):
            nc.sync.dma_start(out=wx_sb[:, k, :], in_=wxT[k])
            nc.sync.dma_start(out=wh_sb[:, k, :], in_=whT[k])
        nc.sync.dma_start(out=bias_sb, in_=bias)
        ident = singles.tile([128, 128], BF16)
        make_identity(nc, ident)

        # ---------------- Phase 1: x-projection ----------------
        p1 = ExitStack()
        xw_pool = p1.enter_context(tc.tile_pool(name="xw", bufs=2))
        stage_pool = p1.enter_context(tc.tile_pool(name="stage", bufs=2))
        px_pool = p1.enter_context(
            tc.tile_pool(name="px", bufs=2, space=bass.MemorySpace.PSUM)
        )
        for w in range(NW):
            xw = xw_pool.tile([128, KC, WIN * B], BF16)
            for k in range(KC):
                nc.sync.dma_start(
                    out=xw[:, k, :], in_=xT[k, :, w * WIN * B : (w + 1) * WIN * B]
                )
            stage = stage_pool.tile([128, WIN, GC, B], BF16)
            for gc in range(GC):
                ps = px_pool.tile([128, WIN * B], F32)
                for k in range(KC):
                    nc.tensor.matmul(
                        ps,
                        wx_sb[:, k, gc * 128 : (gc + 1) * 128],
                        xw[:, k, :],
                        start=(k == 0),
                        stop=(k == KC - 1),
                    )
                # bias add + scatter into stage (strided dest)
                nc.vector.tensor_add(
                    stage[:, :, gc, :],
                    ps.rearrange("p (s b) -> p s b", b=B),
                    bias_sb[:, gc : gc + 1, None].broadcast_to([128, WIN, B]),
                )
            nc.sync.dma_start(
                out=xp_dram[:, w * WIN * 128 : (w + 1) * WIN * 128],
                in_=stage.rearrange("p s g b -> p (s g b)"),
            )

        p1.close()

        # ---------------- Phase 2: recurrence ----------------
        pg_pool = ctx.enter_context(
            tc.tile_pool(name="pg", bufs=4, space=bass.MemorySpace.PSUM)
        )
        xpw_pool = ctx.enter_context(tc.tile_pool(name="xpw", bufs=2))
        ow_pool = ctx.enter_context(tc.tile_pool(name="ow", bufs=2))
        tmp_pool = ctx.enter_context(tc.tile_pool(name="tmp", bufs=4))
        g_pool = ctx.enter_context(tc.tile_pool(name="g", bufs=4))
        state = ctx.enter_context(tc.tile_pool(name="state", bufs=1))

        # One merged set of gate matmuls serves all batch columns (halves the
        # PE weight-reload traffic); the elementwise chain is split into
        # per-half sub-chains that overlap each other on ACT/DVE.
        h_both = state.tile([128, KC * B], BF16)  # cols k*8 + b
        nc.vector.memset(h_both, 0.0)
        h3 = h_both.rearrange("p (k b) -> p k b", b=B)
        halves = []
        for si, (b0, nb) in enumerate(_STREAM_SPLIT):
            c_t = state.tile([128, KC * nb], F32, tag=f"c{si}")
            nc.vector.memset(c_t, 0.0)
            halves.append((si, b0, nb, c_t))

        for w in range(NW):
            xpw = xpw_pool.tile([128, WIN * 128], BF16)
            nc.sync.dma_start(
                out=xpw, in_=xp_dram[:, w * WIN * 128 : (w + 1) * WIN * 128]
            )
            ow = ow_pool.tile([128, WIN * 32], F32)
            for s in range(WIN):
                ow3d = ow[:, s * 32 : (s + 1) * 32].rearrange(
                    "p (h b) -> p h b", b=B
                )
                ps = pg_pool.tile([128, GC * B], F32, tag="ps")
                # Xp preload into PSUM via PE identity pass (h-independent,
                # schedules ahead of the gate matmuls)
                nc.tensor.matmul(
                    ps,
                    ident,
                    xpw[:, s * 128 : (s + 1) * 128],
                    start=True,
                    stop=False,
                    skip_group_check=True,
                )
                for gc in range(GC if not _DBG_SKIP_MM else 1):
                    for k in range(KC):
                        nc.tensor.matmul(
                            ps[:, gc * B : (gc + 1) * B],
                            wh_sb[:, k, gc * 128 : (gc + 1) * 128],
                            h_both[:, k * B : (k + 1) * B],
                            start=False,
                            stop=(k == KC - 1),
                            skip_group_check=True,
                        )
                ps3 = ps.rearrange("p (g b) -> p g b", b=B)
                for si, b0, nb, c_t in halves:
                    osl = ow3d[:, :, b0 : b0 + nb]
                    hsl = h3[:, :, b0 : b0 + nb]
                    if _DBG_SKIP_EW:
                        nc.vector.tensor_copy(out=osl, in_=ps3[:, 0:KC, b0 : b0 + nb])
                        nc.vector.tensor_copy(out=hsl, in_=ps3[:, 0:KC, b0 : b0 + nb])
                        continue
                    a = g_pool.tile([128, GC * nb], F32, tag=f"act{si}")
                    a3 = a.rearrange("p (g b) -> p g b", b=nb)
                    # gate chunks: f 0:4, i 4:8, o 8:12, chat 12:16
                    nc.scalar.activation(
                        a3[:, 0:12, :], ps3[:, 0:12, b0 : b0 + nb], AF.Sigmoid
                    )
                    nc.scalar.activation(
                        a3[:, 12:16, :], ps3[:, 12:16, b0 : b0 + nb], AF.Tanh
                    )
                    q = KC * nb
                    t1 = tmp_pool.tile([128, q], F32, tag=f"t1{si}")
                    t2 = tmp_pool.tile([128, q], F32, tag=f"t2{si}")
                    nc.vector.tensor_mul(t1, a[:, 0:q], c_t)
                    nc.vector.tensor_mul(t2, a[:, q : 2 * q], a[:, 3 * q : 4 * q])
                    nc.vector.tensor_add(c_t, t1, t2)
                    tct = tmp_pool.tile([128, q], F32, tag=f"tct{si}")
                    nc.scalar.activation(tct, c_t, AF.Tanh)
                    nc.vector.tensor_mul(
                        hsl,
                        a3[:, 8:12, :],
                        tct.rearrange("p (k b) -> p k b", b=nb),
                    )
                    nc.vector.tensor_copy(out=osl, in_=hsl)
            nc.sync.dma_start(
                out=out[:, w * WIN * 32 : (w + 1) * WIN * 32], in_=ow
            )

    nc.finalize()
    return nc


def kernel(x, Wf, bf, Wi, bi, Wc, bc, Wo, bo):
    global LAST_RESULTS
    Bfull, S, _ = x.shape
    assert Bfull == NCORES * B

    nc = _cache.get(S)
    if nc is None:
        nc = _cache[S] = _build(S)

    # host-side prep: f, i, o, c gate order
    Wall = np.concatenate([Wf, Wi, Wo, Wc], axis=0)  # [2048, 1024]
    ball = np.concatenate([bf, bi, bo, bc])  # [2048]
    wxT = np.ascontiguousarray(Wall[:, :D].T).reshape(KC, 128, G4)
    whT = np.ascontiguousarray(Wall[:, D:].T).reshape(KC, 128, G4)
    wxT = wxT.astype(ml_dtypes.bfloat16)
    whT = whT.astype(ml_dtypes.bfloat16)
    bias_t = np.ascontiguousarray(ball.reshape(GC, 128).T).astype(np.float32)

    in_maps = []
    for c in range(NCORES):
        xc = x[c * B : (c + 1) * B]  # [B, S, D]
        xTc = np.ascontiguousarray(xc.transpose(2, 1, 0)).reshape(KC, 128, S * B)
        in_maps.append(
            {
                "xT": xTc.astype(ml_dtypes.bfloat16),
                "wxT": wxT,
                "whT": whT,
                "bias": bias_t,
            }
        )

    global LAST_EXEC_WALL
    import time as _time

    _t0 = _time.time()
    res = run_bass_kernel_spmd(nc, in_maps, core_ids=list(range(NCORES)))
    LAST_EXEC_WALL = _time.time() - _t0
    LAST_RESULTS = res

    outputs = np.empty((Bfull, S, H), dtype=np.float32)
    for c in range(NCORES):
        oc = res.results[c]["out"].reshape(128, S, KC, B)  # [p, t, hc, b]
        outputs[c * B : (c + 1) * B] = (
            oc.transpose(3, 1, 2, 0).reshape(B, S, H)
        )
    h_last = np.ascontiguousarray(outputs[:, -1, :])
    return outputs, h_last


# revision 26
# speedup vs baseline: 23.3129x; 23.3129x over previous
"""LSTM (CustomRNNLayer) Trainium2 Bass kernel.

Strategy:
  - Data-parallel over batch: 64 sequences -> 8 cores x B=8.
  - Host pre-transposes x into time-major, feature-on-partition layout
    (xT[k][p, s*8+b]) so the device never transposes anything.
  - Phase 1 (parallel): X-projection GEMM for all timesteps,
    Xp = x @ Wx.T + bias, stored gate-major ([128 gate-partitions,
    t*128 + gc*8 + b] layout) in scratch DRAM.
  - Phase 2 (serial recurrence): per step t, 64 weight-stationary
    matmuls (lhsT = WhT chunk [128,128] bf16 -> FWL, rhs = hT [128,8])
    accumulate G.T in one PSUM tile [128,128]; DVE adds Xp slice; ACT
    sigmoid/tanh (single table set); DVE updates c,h; h written to the
    output window (fp32) and converted to bf16 for the next matmul.
  - Output written as [128, S*32] (cols t*32 + hc*8 + b), reassembled
    on host. h_last = outputs[:, -1].
"""

from contextlib import ExitStack

import ml_dtypes
import numpy as np

import concourse.bass as bass
import concourse.mybir as mybir
import concourse.tile as tile
from concourse import bacc
from concourse.bass_utils import run_bass_kernel_spmd
from concourse.masks import make_identity

NCORES = 8
B = 8  # batch per core
D = 512  # input size
H = 512  # hidden size
G4 = 4 * H  # 2048 gate rows (f, i, o, c order)
KC = D // 128  # 4 contraction chunks
GC = G4 // 128  # 16 gate chunks
WIN = 64  # timesteps per window

F32 = mybir.dt.float32
BF16 = mybir.dt.bfloat16
AF = mybir.ActivationFunctionType

import os as _os

_DBG_SKIP_EW = bool(_os.environ.get("LSTM_DBG_SKIP_EW"))
_DBG_SKIP_MM = bool(_os.environ.get("LSTM_DBG_SKIP_MM"))
_N_STREAMS = int(_os.environ.get("LSTM_STREAMS", "1"))
_STREAM_SPLIT = (
    [(0, B)] if _N_STREAMS == 1 else [(0, B // 2), (B // 2, B // 2)]
)

_cache: dict = {}
LAST_RESULTS = None  # test harness reads exec_time_ns from here
LAST_EXEC_WALL = None  # wall seconds of the SPMD run (test harness)


def _build(S: int) -> bass.Bass:
    NW = S // WIN
    assert S % WIN == 0

    nc = bacc.Bacc("TRN2", debug=False, enable_asserts=False, num_devices=NCORES)
    xT = nc.dram_tensor("xT", [KC, 128, S * B], BF16, kind="ExternalInput").ap()
    wxT = nc.dram_tensor("wxT", [KC, 128, G4], BF16, kind="ExternalInput").ap()
    whT = nc.dram_tensor("whT", [KC, 128, G4], BF16, kind="ExternalInput").ap()
    bias = nc.dram_tensor("bias", [128, GC], F32, kind="ExternalInput").ap()
    out = nc.dram_tensor("out", [128, S * 32], F32, kind="ExternalOutput").ap()

    with tile.TileContext(nc) as tc, ExitStack() as ctx:
        singles = ctx.enter_context(tc.tile_pool(name="singles", bufs=1))
        dram_pool = ctx.enter_context(tc.tile_pool(name="dram", bufs=1, space="DRAM"))

        # scratch DRAM for the x-projection (gate-major step-block layout)
        xp_dram = dram_pool.tile([128, S * 128], BF16)

        # resident weights
        wx_sb = singles.tile([128, KC, G4], BF16)
        wh_sb = singles.tile([128, KC, G4], BF16)
        bias_sb = singles.tile([128, GC], F32)
        for k in range(KC):
            nc.sync.dma_start(out=wx_sb[:, k, :], in_=wxT[k])
            nc.sync.dma_start(out=wh_sb[:, k, :], in_=whT[k])
        nc.sync.dma_start(out=bias_sb, in_=bias)
        ident = singles.tile([128, 128], BF16)
        make_identity(nc, ident)

        # ---------------- Phase 1: x-projection ----------------
        p1 = ExitStack()
        xw_pool = p1.enter_context(tc.tile_pool(name="xw", bufs=2))
        stage_pool = p1.enter_context(tc.tile_pool(name="stage", bufs=2))
        px_pool = p1.enter_context(
            tc.tile_pool(name="px", bufs=2, space=bass.MemorySpace.PSUM)
        )
        for w in range(NW):
            xw = xw_pool.tile([128, KC, WIN * B], BF16)
            for k in range(KC):
                nc.sync.dma_start(
                    out=xw[:, k, :], in_=xT[k, :, w * WIN * B : (w + 1) * WIN * B]
                )
            stage = stage_pool.tile([128, WIN, GC, B], BF16)
            for gc in range(GC):
                ps = px_pool.tile([128, WIN * B], F32)
                for k in range(KC):
                    nc.tensor.matmul(
                        ps,
                        wx_sb[:, k, gc * 128 : (gc + 1) * 128],
                        xw[:, k, :],
                        start=(k == 0),
                        stop=(k == KC - 1),
                    )
                # bias add + scatter into stage (strided dest)
                nc.vector.tensor_add(
                    stage[:, :, gc, :],
                    ps.rearrange("p (s b) -> p s b", b=B),
                    bias_sb[:, gc : gc + 1, None].broadcast_to([128, WIN, B]),
                )
            nc.sync.dma_start(
                out=xp_dram[:, w * WIN * 128 : (w + 1) * WIN * 128],
                in_=stage.rearrange("p s g b -> p (s g b)"),
            )

        p1.close()

        # ---------------- Phase 2: recurrence ----------------
        pg_pool = ctx.enter_context(
            tc.tile_pool(name="pg", bufs=4, space=bass.MemorySpace.PSUM)
        )
        xpw_pool = ctx.enter_context(tc.tile_pool(name="xpw", bufs=2))
        ow_pool = ctx.enter_context(tc.tile_pool(name="ow", bufs=2))
        tmp_pool = ctx.enter_context(tc.tile_pool(name="tmp", bufs=4))
        g_pool = ctx.enter_context(tc.tile_pool(name="g", bufs=4))
        state = ctx.enter_context(tc.tile_pool(name="state", bufs=1))

        # One merged set of gate matmuls serves all batch columns (halves the
        # PE weight-reload traffic); the elementwise chain is split into
        # per-half sub-chains that overlap each other on ACT/DVE.
        h_both = state.tile([128, KC * B], BF16)  # cols k*8 + b
        nc.vector.memset(h_both, 0.0)
        h3 = h_both.rearrange("p (k b) -> p k b", b=B)
        halves = []
        for si, (b0, nb) in enumerate(_STREAM_SPLIT):
            c_t = state.tile([128, KC * nb], F32, tag=f"c{si}")
            nc.vector.memset(c_t, 0.0)
            halves.append((si, b0, nb, c_t))

        xpw_tiles: dict = {}

        def _prefetch_xpw(wi):
            if wi < NW:
                t = xpw_pool.tile([128, WIN * 128], BF16, tag="xpw")
                nc.sync.dma_start(
                    out=t, in_=xp_dram[:, wi * WIN * 128 : (wi + 1) * WIN * 128]
                )
                xpw_tiles[wi] = t

        _prefetch_xpw(0)
        for w in range(NW):
            # prefetch next window's Xp while this window's steps run
            _prefetch_xpw(w + 1)
            xpw = xpw_tiles.pop(w)
            ow = ow_pool.tile([128, WIN * 32], F32)
            for s in range(WIN):
                ow3d = ow[:, s * 32 : (s + 1) * 32].rearrange(
                    "p (h b) -> p h b", b=B
                )
                ps = pg_pool.tile([128, GC * B], F32, tag="ps")
                # Xp preload into PSUM via PE identity pass (h-independent,
                # schedules ahead of the gate matmuls)
                nc.tensor.matmul(
                    ps,
                    ident,
                    xpw[:, s * 128 : (s + 1) * 128],
                    start=True,
                    stop=False,
                    skip_group_check=True,
                )
                for gc in range(GC if not _DBG_SKIP_MM else 1):
                    for k in range(KC):
                        nc.tensor.matmul(
                            ps[:, gc * B : (gc + 1) * B],
                            wh_sb[:, k, gc * 128 : (gc + 1) * 128],
                            h_both[:, k * B : (k + 1) * B],
                            start=False,
                            stop=(k == KC - 1),
                            skip_group_check=True,
                        )
                ps3 = ps.rearrange("p (g b) -> p g b", b=B)
                for si, b0, nb, c_t in halves:
                    osl = ow3d[:, :, b0 : b0 + nb]
                    hsl = h3[:, :, b0 : b0 + nb]
                    if _DBG_SKIP_EW:
                        nc.vector.tensor_copy(out=osl, in_=ps3[:, 0:KC, b0 : b0 + nb])
                        nc.vector.tensor_copy(out=hsl, in_=ps3[:, 0:KC, b0 : b0 + nb])
                        continue
                    a = g_pool.tile([128, GC * nb], F32, tag=f"act{si}")
                    a3 = a.rearrange("p (g b) -> p g b", b=nb)
                    # gate chunks: f 0:4, i 4:8, o 8:12, chat 12:16
                    nc.scalar.activation(
                        a3[:, 0:12, :], ps3[:, 0:12, b0 : b0 + nb], AF.Sigmoid
                    )
                    nc.scalar.activation(
                        a3[:, 12:16, :], ps3[:, 12:16, b0 : b0 + nb], AF.Tanh
                    )
                    q = KC * nb
                    t1 = tmp_pool.tile([128, q], F32, tag=f"t1{si}")
                    t2 = tmp_pool.tile([128, q], F32, tag=f"t2{si}")
                    nc.vector.tensor_mul(t1, a[:, 0:q], c_t)
                    nc.vector.tensor_mul(t2, a[:, q : 2 * q], a[:, 3 * q : 4 * q])
                    nc.vector.tensor_add(c_t, t1, t2)
                    tct = tmp_pool.tile([128, q], F32, tag=f"tct{si}")
                    nc.scalar.activation(tct, c_t, AF.Tanh)
                    nc.vector.tensor_mul(
                        hsl,
                        a3[:, 8:12, :],
                        tct.rearrange("p (k b) -> p k b", b=nb),
                    )
                    nc.vector.tensor_copy(out=osl, in_=hsl)
            # output store on the gpsimd DMA queue so it never delays the
            # next window's Xp prefetch on the sync queue
            nc.gpsimd.dma_start(
                out=out[:, w * WIN * 32 : (w + 1) * WIN * 32], in_=ow
            )

    nc.finalize()
    return nc


def kernel(x, Wf, bf, Wi, bi, Wc, bc, Wo, bo):
    global LAST_RESULTS
    Bfull, S, _ = x.shape
    assert Bfull == NCORES * B

    nc = _cache.get(S)
    if nc is None:
        nc = _cache[S] = _build(S)

    # host-side prep: f, i, o, c gate order
    Wall = np.concatenate([Wf, Wi, Wo, Wc], axis=0)  # [2048, 1024]
    ball = np.concatenate([bf, bi, bo, bc])  # [2048]
    wxT = np.ascontiguousarray(Wall[:, :D].T).reshape(KC, 128, G4)
    whT = np.ascontiguousarray(Wall[:, D:].T).reshape(KC, 128, G4)
    wxT = wxT.astype(ml_dtypes.bfloat16)
    whT = whT.astype(ml_dtypes.bfloat16)
    bias_t = np.ascontiguousarray(ball.reshape(GC, 128).T).astype(np.float32)

    in_maps = []
    for c in range(NCORES):
        xc = x[c * B : (c + 1) * B]  # [B, S, D]
        xTc = np.ascontiguousarray(xc.transpose(2, 1, 0)).reshape(KC, 128, S * B)
        in_maps.append(
            {
                "xT": xTc.astype(ml_dtypes.bfloat16),
                "wxT": wxT,
                "whT": whT,
                "bias": bias_t,
            }
        )

    global LAST_EXEC_WALL
    import time as _time

    _t0 = _time.time()
    res = run_bass_kernel_spmd(nc, in_maps, core_ids=list(range(NCORES)))
    LAST_EXEC_WALL = _time.time() - _t0
    LAST_RESULTS = res

    outputs = np.empty((Bfull, S, H), dtype=np.float32)
    for c in range(NCORES):
        oc = res.results[c]["out"].reshape(128, S, KC, B)  # [p, t, hc, b]
        outputs[c * B : (c + 1) * B] = (
            oc.transpose(3, 1, 2, 0).reshape(B, S, H)
        )
    h_last = np.ascontiguousarray(outputs[:, -1, :])
    return outputs, h_last
